# revision 4
# baseline (speedup 1.0000x reference)
"""GCN message-passing (nn_Discriminator) on 8 Trainium2 NeuronCores.

Algorithm: per layer, h_new = (A@h)@Wm_h + h@Ws + (A@edge_attr_seg)@Wm_e + b
(the per-edge matmul commutes with segment_sum). A@h is computed per core
(nodes sharded by dst) as one-hot scatter matmuls on TensorE over edge tiles
sorted by dst window, with h rows fetched by dma_gather. Between layers the
node features are AllGathered (bf16) so every core can gather any row.

Sharding: nodes (and edges by dst) across 8 cores; weights replicated.
"""
import numpy as np
import ml_dtypes

bf16 = ml_dtypes.bfloat16

N_NODES = 50000
N_EDGES = 800000
N_CORES = 8
NPC = N_NODES // N_CORES          # 6250 real nodes per core
P = 128
NWIN = (NPC + P - 1) // P         # 49 windows
NL = NWIN * P                     # 6272 padded nodes per core
NPOS = NL * N_CORES               # 50176 padded gather-source rows
HALF = 32768                      # int16 gather index split
CH = 64                           # gather chunk, tiles per dma_gather call
DIMS = [32, 64, 128, 128, 128, 128]
DIN_S = DIMS[:5]
DOUT = DIMS[1:]

_cache = {}


def _build_schedule(src, dst):
    src = np.asarray(src).astype(np.int64)
    dst = np.asarray(dst).astype(np.int64)
    rank = dst // NPC
    dloc = dst % NPC
    w = dloc // P
    spos = (src // NPC) * NL + (src % NPC)
    half = (spos >= HALF).astype(np.int64)

    order = np.lexsort((spos, half, w, rank))
    rs, ws, hs = rank[order], w[order], half[order]
    counts = np.zeros((N_CORES, NWIN, 2), np.int64)
    np.add.at(counts, (rs, ws, hs), 1)
    ntiles = (-(-counts // P)).max(axis=0)       # [NWIN, 2]

    sched = []
    for wi in range(NWIN):
        for h in (0, 1):
            if ntiles[wi, h] > 0:
                sched.append((wi, h, int(ntiles[wi, h])))

    key = (rs * NWIN + ws) * 2 + hs
    boundaries = np.searchsorted(key, np.arange(N_CORES * NWIN * 2 + 1))
    per_core = []
    for c in range(N_CORES):
        posA, posB, dstw_all, eidx_all = [], [], [], []
        for (wi, h, nt) in sched:
            k = (c * NWIN + wi) * 2 + h
            lo, hi = boundaries[k], boundaries[k + 1]
            e = order[lo:hi]
            npad = nt * P - (hi - lo)
            ps = np.concatenate([spos[e] - (HALF if h else 0),
                                 np.zeros(npad, np.int64)])
            dl = np.concatenate([dloc[e] - wi * P, -np.ones(npad, np.int64)])
            ei = np.concatenate([e, -np.ones(npad, np.int64)])
            (posA if h == 0 else posB).append(ps)
            dstw_all.append(dl)
            eidx_all.append(ei)
        per_core.append(dict(
            posA=np.concatenate(posA).astype(np.int32),
            posB=np.concatenate(posB).astype(np.int32),
            dstw=np.concatenate(dstw_all).astype(np.int32),
            eidx=np.concatenate(eidx_all).astype(np.int64),
        ))
    return sched, per_core


def _wrap_idx(pos):
    n = len(pos)
    img = np.zeros((128, max(n // 16, 1)), np.int16)
    if n:
        img[np.arange(n) % 16, np.arange(n) // 16] = pos.astype(np.int16)
        for g in range(1, 8):
            img[g * 16:(g + 1) * 16] = img[:16]
    return img


def _build_program(sched, TA, TB):
    from concourse import mybir, bacc
    import concourse.tile as tile

    T = sum(nt for _, _, nt in sched)
    nc = bacc.Bacc("TRN2", target_bir_lowering=False, debug=False,
                   num_devices=N_CORES)
    dt = mybir.dt

    x_pos = nc.dram_tensor("x_pos", [NPOS, 128], dt.bfloat16, kind="ExternalInput")
    xT_in = nc.dram_tensor("xT", [32, NL], dt.bfloat16, kind="ExternalInput")
    idxA_d = nc.dram_tensor("idxA", [128, TA * 8], dt.int16, kind="ExternalInput")
    idxB_d = nc.dram_tensor("idxB", [128, TB * 8], dt.int16, kind="ExternalInput")
    dstw_d = nc.dram_tensor("dstw", [128, T], dt.bfloat16, kind="ExternalInput")
    ea_d = nc.dram_tensor("eaimg", [128, T * 8], dt.bfloat16, kind="ExternalInput")
    iota_d = nc.dram_tensor("iota", [128, 128], dt.bfloat16, kind="ExternalInput")
    ident_d = nc.dram_tensor("ident", [128, 128], dt.bfloat16, kind="ExternalInput")
    wmh_d, ws_d, wme_d, b_d = [], [], [], []
    for l in range(5):
        wmh_d.append(nc.dram_tensor(f"Wmh{l}", [DIN_S[l], DOUT[l]], dt.bfloat16, kind="ExternalInput"))
        ws_d.append(nc.dram_tensor(f"Wsb{l}", [DIN_S[l], DOUT[l]], dt.bfloat16, kind="ExternalInput"))
        wme_d.append(nc.dram_tensor(f"Wme{l}", [8, DOUT[l]], dt.bfloat16, kind="ExternalInput"))
        b_d.append(nc.dram_tensor(f"bb{l}", [DOUT[l], 1], dt.float32, kind="ExternalInput"))
    wc_d = nc.dram_tensor("Wcb", [128, 1], dt.bfloat16, kind="ExternalInput")
    bc_d = nc.dram_tensor("bcb", [128, 1], dt.float32, kind="ExternalInput")
    out_d = nc.dram_tensor("out", [NL, 1], dt.float32, kind="ExternalOutput")

    with tile.TileContext(nc) as tc:
        with tc.tile_pool(name="consts", bufs=1) as consts, \
             tc.tile_pool(name="gxa", bufs=2) as gxa_pool, \
             tc.tile_pool(name="gxb", bufs=2) as gxb_pool, \
             tc.tile_pool(name="rp", bufs=4) as rpool, \
             tc.tile_pool(name="wk", bufs=3) as wk, \
             tc.tile_pool(name="hT", bufs=2) as hT_pool, \
             tc.tile_pool(name="ahp", bufs=2, space="PSUM") as ah_ps, \
             tc.tile_pool(name="aep", bufs=2, space="PSUM") as ae_ps, \
             tc.tile_pool(name="hnp", bufs=2, space="PSUM") as hn_ps, \
             tc.tile_pool(name="trp", bufs=2, space="PSUM") as tr_ps, \
             tc.tile_pool(name="dram", bufs=1, space="DRAM") as dram:

            # ---- resident constants ----
            idxA = consts.tile([128, TA * 8], dt.int16)
            idxB = consts.tile([128, TB * 8], dt.int16)
            dstw = consts.tile([128, T], dt.bfloat16)
            eaim = consts.tile([128, T * 8], dt.bfloat16)
            iota = consts.tile([128, 128], dt.bfloat16)
            ident = consts.tile([128, 128], dt.bfloat16)
            nc.sync.dma_start(out=idxA[:], in_=idxA_d[:])
            nc.sync.dma_start(out=idxB[:], in_=idxB_d[:])
            nc.sync.dma_start(out=dstw[:], in_=dstw_d[:])
            nc.sync.dma_start(out=eaim[:], in_=ea_d[:])
            nc.sync.dma_start(out=iota[:], in_=iota_d[:])
            nc.sync.dma_start(out=ident[:], in_=ident_d[:])
            wmh, wsb, wme, bb = [], [], [], []
            for l in range(5):
                t1 = consts.tile([DIN_S[l], DOUT[l]], dt.bfloat16, tag=f"wmh{l}")
                t2 = consts.tile([DIN_S[l], DOUT[l]], dt.bfloat16, tag=f"wsb{l}")
                t3 = consts.tile([8, DOUT[l]], dt.bfloat16, tag=f"wme{l}")
                t4 = consts.tile([DOUT[l], 1], dt.float32, tag=f"bb{l}")
                nc.sync.dma_start(out=t1[:], in_=wmh_d[l][:])
                nc.sync.dma_start(out=t2[:], in_=ws_d[l][:])
                nc.sync.dma_start(out=t3[:], in_=wme_d[l][:])
                nc.sync.dma_start(out=t4[:], in_=b_d[l][:])
                wmh.append(t1); wsb.append(t2); wme.append(t3); bb.append(t4)
            wc = consts.tile([128, 1], dt.bfloat16)
            bcb = consts.tile([128, 1], dt.float32)
            nc.sync.dma_start(out=wc[:], in_=wc_d[:])
            nc.sync.dma_start(out=bcb[:], in_=bc_d[:])
            xT = consts.tile([32, NL], dt.bfloat16)
            nc.sync.dma_start(out=xT[:], in_=xT_in[:])
            aeT = consts.tile([8, NL], dt.bfloat16)

            # per-window tile counts (both halves)
            wtiles = [0] * NWIN
            for (wi, h, nt) in sched:
                wtiles[wi] += nt

            cc_in = [dram.tile([NL, 128], dt.bfloat16, tag=f"ccin{l}", name=f"ccin{l}")
                     for l in range(4)]
            cc_out = [dram.tile([NPOS, 128], dt.bfloat16, tag=f"ccout{l}", name=f"ccout{l}")
                      for l in range(4)]

            hT_cur = xT
            for l in range(5):
                din, dout = DIN_S[l], DOUT[l]
                src_dram = x_pos if l == 0 else cc_out[l - 1]
                stream_src = {"A": src_dram[:, :], "B": src_dram[HALF:, :]}
                stream_idx = {"A": idxA, "B": idxB}
                stream_T = {"A": TA, "B": TB}
                gx_pool = {"A": gxa_pool, "B": gxb_pool}
                gx_tiles = {"A": {}, "B": {}}

                def slot(S, j, l=l):
                    ch = j // CH
                    tl = gx_tiles[S]
                    if ch not in tl:
                        nt = min(CH, stream_T[S] - ch * CH)
                        g = gx_pool[S].tile([128, nt, 128], dt.bfloat16, tag="gx" + S)
                        nc.gpsimd.dma_gather(
                            out_ap=g[:],
                            in_ap=stream_src[S],
                            idxs_ap=stream_idx[S][:, ch * CH * 8:(ch * CH + nt) * 8],
                            num_idxs=nt * 128,
                            num_idxs_reg=nt * 128,
                            elem_size=128,
                            single_packet=False,
                        )
                        tl[ch] = g
                    return tl[ch][:, j % CH, :]

                if l == 0:
                    hT_next = hT_pool.tile([128, NL], dt.bfloat16, tag="hT")
                    nc.vector.memset(hT_next[64:128, :], 0)
                else:
                    hT_next = hT_pool.tile([128, NL], dt.bfloat16, tag="hT")

                a_ctr = b_ctr = 0
                t_global = 0
                cur_w = -1
                done_in_w = 0
                for (wi, h, nt) in sched:
                    if wi != cur_w:
                        cur_w = wi
                        done_in_w = 0
                        ah = ah_ps.tile([din, 128], dt.float32, tag="ah", space="PSUM")
                        if l == 0:
                            ae = ae_ps.tile([8, 128], dt.float32, tag="ae", space="PSUM")
                    for j in range(nt):
                        if h == 0:
                            xs = slot("A", a_ctr); a_ctr += 1
                        else:
                            xs = slot("B", b_ctr); b_ctr += 1
                        R = rpool.tile([128, 128], dt.bfloat16, tag="R")
                        nc.vector.tensor_tensor(
                            out=R[:],
                            in0=dstw[:, t_global:t_global + 1].to_broadcast([128, 128]),
                            in1=iota[:],
                            op=mybir.AluOpType.is_equal,
                        )
                        first = done_in_w == 0
                        last = done_in_w == wtiles[wi] - 1
                        nc.tensor.matmul(out=ah[:], lhsT=xs[:, :din], rhs=R[:],
                                         start=first, stop=last)
                        if l == 0:
                            nc.tensor.matmul(
                                out=ae[:], lhsT=eaim[:, t_global * 8:(t_global + 1) * 8],
                                rhs=R[:], start=first, stop=last)
                        done_in_w += 1
                        t_global += 1

                    if done_in_w == wtiles[wi]:
                        # ---- window epilogue ----
                        wsl = slice(wi * P, (wi + 1) * P)
                        ah_sb = wk.tile([din, 128], dt.bfloat16, tag="ah_sb")
                        nc.vector.tensor_copy(out=ah_sb[:], in_=ah[:])
                        if l == 0:
                            nc.vector.tensor_copy(out=aeT[:, wsl], in_=ae[:])
                        hn = hn_ps.tile([dout, 128], dt.float32, tag="hn", space="PSUM")
                        nc.tensor.matmul(out=hn[:], lhsT=wmh[l][:], rhs=ah_sb[:],
                                         start=True, stop=False)
                        nc.tensor.matmul(out=hn[:], lhsT=wsb[l][:],
                                         rhs=hT_cur[:din, wsl], start=False, stop=False)
                        nc.tensor.matmul(out=hn[:], lhsT=wme[l][:], rhs=aeT[:, wsl],
                                         start=False, stop=True)
                        nc.vector.tensor_scalar(
                            out=hT_next[:dout, wsl], in0=hn[:],
                            scalar1=bb[l][:], scalar2=0.0,
                            op0=mybir.AluOpType.add, op1=mybir.AluOpType.max)
                        if l < 4:
                            tr = tr_ps.tile([128, 128], dt.bfloat16, tag="tr", space="PSUM")
                            nc.tensor.transpose(out=tr[:], in_=hT_next[:, wsl], identity=ident[:])
                            rows = wk.tile([128, 128], dt.bfloat16, tag="rows")
                            nc.vector.tensor_copy(out=rows[:], in_=tr[:])
                            nc.sync.dma_start(out=cc_in[l][wsl, :], in_=rows[:])
                        else:
                            hd = tr_ps.tile([128, 1], dt.float32, tag="tr", space="PSUM")
                            nc.tensor.matmul(out=hd[:], lhsT=hT_next[:, wsl], rhs=wc[:],
                                             start=True, stop=True)
                            hd_sb = wk.tile([128, 1], dt.float32, tag="hd_sb")
                            nc.vector.tensor_scalar(
                                out=hd_sb[:], in0=hd[:], scalar1=bcb[:], scalar2=None,
                                op0=mybir.AluOpType.add)
                            nc.sync.dma_start(out=out_d[wsl, :], in_=hd_sb[:])

                if l < 4:
                    nc.gpsimd.collective_compute(
                        "AllGather",
                        mybir.AluOpType.bypass,
                        replica_groups=[list(range(N_CORES))],
                        ins=[cc_in[l].opt()],
                        outs=[cc_out[l].opt()],
                    )
                hT_cur = hT_next

    nc.finalize()
    return nc


def _to_pos_layout(arr, width=128):
    d = arr.shape[1]
    out = np.zeros((NPOS, width), arr.dtype)
    r = np.arange(N_NODES)
    out[(r // NPC) * NL + (r % NPC), :d] = arr
    return out


def kernel(**inputs):
    from concourse.bass_utils import run_bass_kernel_spmd

    src = np.asarray(inputs["src"]).astype(np.int64)
    dst = np.asarray(inputs["dst"]).astype(np.int64)
    sched, per_core = _build_schedule(src, dst)
    T = sum(nt for _, _, nt in sched)
    TA = sum(nt for _, h, nt in sched if h == 0)
    TB = T - TA

    key = ("v1", T, TA, tuple((w, h, nt) for w, h, nt in sched))
    if key not in _cache:
        _cache.clear()
        _cache[key] = _build_program(sched, TA, TB)
    nc = _cache[key]

    x = np.asarray(inputs["x"], np.float32)
    ea = np.asarray(inputs["edge_attr"], np.float32)
    x_pos = _to_pos_layout(x.astype(bf16))
    iota = np.tile(np.arange(128, dtype=np.float32)[None, :], (128, 1)).astype(bf16)
    ident = np.eye(128, dtype=np.float32).astype(bf16)

    shared = {"x_pos": x_pos, "iota": iota, "ident": ident}
    for l in range(5):
        Wm = np.asarray(inputs[f"Wm{l}"], np.float32)
        shared[f"Wmh{l}"] = Wm[:DIN_S[l]].astype(bf16)
        shared[f"Wme{l}"] = Wm[DIN_S[l]:].astype(bf16)
        shared[f"Wsb{l}"] = np.asarray(inputs[f"Ws{l}"], np.float32).astype(bf16)
        shared[f"bb{l}"] = np.asarray(inputs[f"b{l}"], np.float32).reshape(-1, 1)
    shared["Wcb"] = np.asarray(inputs["Wc"], np.float32).astype(bf16)
    shared["bcb"] = np.full((128, 1), np.asarray(inputs["bc"], np.float32).reshape(-1)[0], np.float32)

    in_maps = []
    for c in range(N_CORES):
        pc = per_core[c]
        dstw_img = pc["dstw"].reshape(T, 128).T.astype(np.float32).astype(bf16)
        ei = pc["eidx"]
        eav = np.zeros((T * 128, 8), np.float32)
        m = ei >= 0
        eav[m] = ea[ei[m]]
        ea_img = np.ascontiguousarray(
            eav.reshape(T, 128, 8).transpose(1, 0, 2).reshape(128, T * 8)).astype(bf16)
        xT_img = np.ascontiguousarray(x_pos[c * NL:(c + 1) * NL, :32].T)
        in_maps.append({
            **shared,
            "idxA": _wrap_idx(pc["posA"]),
            "idxB": _wrap_idx(pc["posB"]),
            "dstw": dstw_img,
            "eaimg": ea_img,
            "xT": xT_img,
        })

    res = run_bass_kernel_spmd(nc, in_maps, core_ids=list(range(N_CORES)))
    out = np.concatenate([res.results[c]["out"][:NPC] for c in range(N_CORES)], axis=0)
    return out.astype(np.float32)


# revision 5
# speedup vs baseline: 1.5747x; 1.5747x over previous
"""GCN message-passing (nn_Discriminator) on 8 Trainium2 NeuronCores.

Algorithm: per layer, h_new = (A@h)@Wm_h + h@Ws + (A@edge_attr_seg)@Wm_e + b
(the per-edge matmul commutes with segment_sum). A@h is computed per core
(nodes sharded by dst) as one-hot scatter matmuls on TensorE over edge tiles
sorted by dst window, with h rows fetched by dma_gather. Between layers the
node features are AllGathered (bf16) so every core can gather any row.

Sharding: nodes (and edges by dst) across 8 cores; weights replicated.
"""
import numpy as np
import ml_dtypes

bf16 = ml_dtypes.bfloat16

N_NODES = 50000
N_EDGES = 800000
N_CORES = 8
NPC = N_NODES // N_CORES          # 6250 real nodes per core
P = 128
NWIN = (NPC + P - 1) // P         # 49 windows
NL = NWIN * P                     # 6272 padded nodes per core
NPOS = NL * N_CORES               # 50176 padded gather-source rows
HALF = 32768                      # int16 gather index split
CH = 64                           # gather chunk, tiles per dma_gather call
DIMS = [32, 64, 128, 128, 128, 128]
DIN_S = DIMS[:5]
DOUT = DIMS[1:]

_cache = {}


def _build_schedule(src, dst):
    src = np.asarray(src).astype(np.int64)
    dst = np.asarray(dst).astype(np.int64)
    rank = dst // NPC
    dloc = dst % NPC
    w = dloc // P
    spos = (src // NPC) * NL + (src % NPC)
    half = (spos >= HALF).astype(np.int64)

    order = np.lexsort((spos, half, w, rank))
    rs, ws, hs = rank[order], w[order], half[order]
    counts = np.zeros((N_CORES, NWIN, 2), np.int64)
    np.add.at(counts, (rs, ws, hs), 1)
    ntiles = (-(-counts // P)).max(axis=0)       # [NWIN, 2]

    sched = []
    for wi in range(NWIN):
        for h in (0, 1):
            if ntiles[wi, h] > 0:
                sched.append((wi, h, int(ntiles[wi, h])))

    key = (rs * NWIN + ws) * 2 + hs
    boundaries = np.searchsorted(key, np.arange(N_CORES * NWIN * 2 + 1))
    per_core = []
    for c in range(N_CORES):
        posA, posB, dstw_all, eidx_all = [], [], [], []
        for (wi, h, nt) in sched:
            k = (c * NWIN + wi) * 2 + h
            lo, hi = boundaries[k], boundaries[k + 1]
            e = order[lo:hi]
            npad = nt * P - (hi - lo)
            ps = np.concatenate([spos[e] - (HALF if h else 0),
                                 np.zeros(npad, np.int64)])
            dl = np.concatenate([dloc[e] - wi * P, -np.ones(npad, np.int64)])
            ei = np.concatenate([e, -np.ones(npad, np.int64)])
            (posA if h == 0 else posB).append(ps)
            dstw_all.append(dl)
            eidx_all.append(ei)
        per_core.append(dict(
            posA=np.concatenate(posA).astype(np.int32),
            posB=np.concatenate(posB).astype(np.int32),
            dstw=np.concatenate(dstw_all).astype(np.int32),
            eidx=np.concatenate(eidx_all).astype(np.int64),
        ))
    return sched, per_core


def _wrap_idx(pos):
    n = len(pos)
    img = np.zeros((128, max(n // 16, 1)), np.int16)
    if n:
        img[np.arange(n) % 16, np.arange(n) // 16] = pos.astype(np.int16)
        for g in range(1, 8):
            img[g * 16:(g + 1) * 16] = img[:16]
    return img


def _build_program(sched, TA, TB):
    from concourse import mybir, bacc
    import concourse.tile as tile

    T = sum(nt for _, _, nt in sched)
    nc = bacc.Bacc("TRN2", target_bir_lowering=False, debug=False,
                   num_devices=N_CORES, num_swdge_queues=4)
    dt = mybir.dt

    x_pos = nc.dram_tensor("x_pos", [NPOS, 128], dt.bfloat16, kind="ExternalInput")
    xT_in = nc.dram_tensor("xT", [32, NL], dt.bfloat16, kind="ExternalInput")
    idxA_d = nc.dram_tensor("idxA", [128, TA * 8], dt.int16, kind="ExternalInput")
    idxB_d = nc.dram_tensor("idxB", [128, TB * 8], dt.int16, kind="ExternalInput")
    dstw_d = nc.dram_tensor("dstw", [128, T], dt.bfloat16, kind="ExternalInput")
    ea_d = nc.dram_tensor("eaimg", [128, T * 8], dt.bfloat16, kind="ExternalInput")
    iota_d = nc.dram_tensor("iota", [128, 128], dt.bfloat16, kind="ExternalInput")
    ident_d = nc.dram_tensor("ident", [128, 128], dt.bfloat16, kind="ExternalInput")
    wmh_d, ws_d, wme_d, b_d = [], [], [], []
    for l in range(5):
        wmh_d.append(nc.dram_tensor(f"Wmh{l}", [DIN_S[l], DOUT[l]], dt.bfloat16, kind="ExternalInput"))
        ws_d.append(nc.dram_tensor(f"Wsb{l}", [DIN_S[l], DOUT[l]], dt.bfloat16, kind="ExternalInput"))
        wme_d.append(nc.dram_tensor(f"Wme{l}", [8, DOUT[l]], dt.bfloat16, kind="ExternalInput"))
        b_d.append(nc.dram_tensor(f"bb{l}", [DOUT[l], 1], dt.float32, kind="ExternalInput"))
    wc_d = nc.dram_tensor("Wcb", [128, 1], dt.bfloat16, kind="ExternalInput")
    bc_d = nc.dram_tensor("bcb", [128, 1], dt.float32, kind="ExternalInput")
    out_d = nc.dram_tensor("out", [NL, 1], dt.float32, kind="ExternalOutput")

    with tile.TileContext(nc) as tc:
        with tc.tile_pool(name="consts", bufs=1) as consts, \
             tc.tile_pool(name="gxa", bufs=2) as gxa_pool, \
             tc.tile_pool(name="gxb", bufs=2) as gxb_pool, \
             tc.tile_pool(name="rp", bufs=4) as rpool, \
             tc.tile_pool(name="wk", bufs=3) as wk, \
             tc.tile_pool(name="hT", bufs=2) as hT_pool, \
             tc.tile_pool(name="ahp", bufs=2, space="PSUM") as ah_ps, \
             tc.tile_pool(name="aep", bufs=2, space="PSUM") as ae_ps, \
             tc.tile_pool(name="hnp", bufs=2, space="PSUM") as hn_ps, \
             tc.tile_pool(name="trp", bufs=2, space="PSUM") as tr_ps, \
             tc.tile_pool(name="dram", bufs=1, space="DRAM") as dram:

            # ---- resident constants ----
            idxA = consts.tile([128, TA * 8], dt.int16)
            idxB = consts.tile([128, TB * 8], dt.int16)
            dstw = consts.tile([128, T], dt.bfloat16)
            eaim = consts.tile([128, T * 8], dt.bfloat16)
            iota = consts.tile([128, 128], dt.bfloat16)
            ident = consts.tile([128, 128], dt.bfloat16)
            nc.sync.dma_start(out=idxA[:], in_=idxA_d[:])
            nc.sync.dma_start(out=idxB[:], in_=idxB_d[:])
            nc.sync.dma_start(out=dstw[:], in_=dstw_d[:])
            nc.sync.dma_start(out=eaim[:], in_=ea_d[:])
            nc.sync.dma_start(out=iota[:], in_=iota_d[:])
            nc.sync.dma_start(out=ident[:], in_=ident_d[:])
            wmh, wsb, wme, bb = [], [], [], []
            for l in range(5):
                t1 = consts.tile([DIN_S[l], DOUT[l]], dt.bfloat16, tag=f"wmh{l}")
                t2 = consts.tile([DIN_S[l], DOUT[l]], dt.bfloat16, tag=f"wsb{l}")
                t3 = consts.tile([8, DOUT[l]], dt.bfloat16, tag=f"wme{l}")
                t4 = consts.tile([DOUT[l], 1], dt.float32, tag=f"bb{l}")
                nc.sync.dma_start(out=t1[:], in_=wmh_d[l][:])
                nc.sync.dma_start(out=t2[:], in_=ws_d[l][:])
                nc.sync.dma_start(out=t3[:], in_=wme_d[l][:])
                nc.sync.dma_start(out=t4[:], in_=b_d[l][:])
                wmh.append(t1); wsb.append(t2); wme.append(t3); bb.append(t4)
            wc = consts.tile([128, 1], dt.bfloat16)
            bcb = consts.tile([128, 1], dt.float32)
            nc.sync.dma_start(out=wc[:], in_=wc_d[:])
            nc.sync.dma_start(out=bcb[:], in_=bc_d[:])
            xT = consts.tile([32, NL], dt.bfloat16)
            nc.sync.dma_start(out=xT[:], in_=xT_in[:])
            aeT = consts.tile([8, NL], dt.bfloat16)

            # per-window tile counts (both halves)
            wtiles = [0] * NWIN
            for (wi, h, nt) in sched:
                wtiles[wi] += nt

            cc_in = [dram.tile([NL, 128], dt.bfloat16, tag=f"ccin{l}", name=f"ccin{l}")
                     for l in range(4)]
            cc_out = [dram.tile([NPOS, 128], dt.bfloat16, tag=f"ccout{l}", name=f"ccout{l}")
                      for l in range(4)]

            hT_cur = xT
            for l in range(5):
                din, dout = DIN_S[l], DOUT[l]
                src_dram = x_pos if l == 0 else cc_out[l - 1]
                stream_src = {"A": src_dram[:, :], "B": src_dram[HALF:, :]}
                stream_idx = {"A": idxA, "B": idxB}
                stream_T = {"A": TA, "B": TB}
                gx_pool = {"A": gxa_pool, "B": gxb_pool}
                gx_tiles = {"A": {}, "B": {}}
                qrr = [0]

                def slot(S, j, l=l):
                    ch = j // CH
                    tl = gx_tiles[S]
                    if ch not in tl:
                        nt = min(CH, stream_T[S] - ch * CH)
                        g = gx_pool[S].tile([128, nt, 128], dt.bfloat16, tag="gx" + S)
                        nc.gpsimd.dma_gather(
                            out_ap=g[:],
                            in_ap=stream_src[S],
                            idxs_ap=stream_idx[S][:, ch * CH * 8:(ch * CH + nt) * 8],
                            num_idxs=nt * 128,
                            num_idxs_reg=nt * 128,
                            elem_size=128,
                            single_packet=False,
                            queue_num=qrr[0],
                        )
                        qrr[0] = (qrr[0] + 1) % 4
                        tl[ch] = g
                    return tl[ch][:, j % CH, :]

                if l == 0:
                    hT_next = hT_pool.tile([128, NL], dt.bfloat16, tag="hT")
                    nc.vector.memset(hT_next[64:128, :], 0)
                else:
                    hT_next = hT_pool.tile([128, NL], dt.bfloat16, tag="hT")

                a_ctr = b_ctr = 0
                t_global = 0
                cur_w = -1
                done_in_w = 0
                for (wi, h, nt) in sched:
                    if wi != cur_w:
                        cur_w = wi
                        done_in_w = 0
                        ah = ah_ps.tile([din, 128], dt.float32, tag="ah", space="PSUM")
                        if l == 0:
                            ae = ae_ps.tile([8, 128], dt.float32, tag="ae", space="PSUM")
                    for j in range(nt):
                        if h == 0:
                            xs = slot("A", a_ctr); a_ctr += 1
                        else:
                            xs = slot("B", b_ctr); b_ctr += 1
                        R = rpool.tile([128, 128], dt.bfloat16, tag="R")
                        nc.vector.tensor_tensor(
                            out=R[:],
                            in0=dstw[:, t_global:t_global + 1].to_broadcast([128, 128]),
                            in1=iota[:],
                            op=mybir.AluOpType.is_equal,
                        )
                        first = done_in_w == 0
                        last = done_in_w == wtiles[wi] - 1
                        nc.tensor.matmul(out=ah[:], lhsT=xs[:, :din], rhs=R[:],
                                         start=first, stop=last)
                        if l == 0:
                            nc.tensor.matmul(
                                out=ae[:], lhsT=eaim[:, t_global * 8:(t_global + 1) * 8],
                                rhs=R[:], start=first, stop=last)
                        done_in_w += 1
                        t_global += 1

                    if done_in_w == wtiles[wi]:
                        # ---- window epilogue ----
                        wsl = slice(wi * P, (wi + 1) * P)
                        ah_sb = wk.tile([din, 128], dt.bfloat16, tag="ah_sb")
                        nc.vector.tensor_copy(out=ah_sb[:], in_=ah[:])
                        if l == 0:
                            nc.vector.tensor_copy(out=aeT[:, wsl], in_=ae[:])
                        hn = hn_ps.tile([dout, 128], dt.float32, tag="hn", space="PSUM")
                        nc.tensor.matmul(out=hn[:], lhsT=wmh[l][:], rhs=ah_sb[:],
                                         start=True, stop=False)
                        nc.tensor.matmul(out=hn[:], lhsT=wsb[l][:],
                                         rhs=hT_cur[:din, wsl], start=False, stop=False)
                        nc.tensor.matmul(out=hn[:], lhsT=wme[l][:], rhs=aeT[:, wsl],
                                         start=False, stop=True)
                        nc.vector.tensor_scalar(
                            out=hT_next[:dout, wsl], in0=hn[:],
                            scalar1=bb[l][:], scalar2=0.0,
                            op0=mybir.AluOpType.add, op1=mybir.AluOpType.max)
                        if l < 4:
                            tr = tr_ps.tile([128, 128], dt.bfloat16, tag="tr", space="PSUM")
                            nc.tensor.transpose(out=tr[:], in_=hT_next[:, wsl], identity=ident[:])
                            rows = wk.tile([128, 128], dt.bfloat16, tag="rows")
                            nc.vector.tensor_copy(out=rows[:], in_=tr[:])
                            nc.sync.dma_start(out=cc_in[l][wsl, :], in_=rows[:])
                        else:
                            hd = tr_ps.tile([128, 1], dt.float32, tag="tr", space="PSUM")
                            nc.tensor.matmul(out=hd[:], lhsT=hT_next[:, wsl], rhs=wc[:],
                                             start=True, stop=True)
                            hd_sb = wk.tile([128, 1], dt.float32, tag="hd_sb")
                            nc.vector.tensor_scalar(
                                out=hd_sb[:], in0=hd[:], scalar1=bcb[:], scalar2=None,
                                op0=mybir.AluOpType.add)
                            nc.sync.dma_start(out=out_d[wsl, :], in_=hd_sb[:])

                if l < 4:
                    nc.gpsimd.collective_compute(
                        "AllGather",
                        mybir.AluOpType.bypass,
                        replica_groups=[list(range(N_CORES))],
                        ins=[cc_in[l].opt()],
                        outs=[cc_out[l].opt()],
                    )
                hT_cur = hT_next

    nc.finalize()
    return nc


def _to_pos_layout(arr, width=128):
    d = arr.shape[1]
    out = np.zeros((NPOS, width), arr.dtype)
    r = np.arange(N_NODES)
    out[(r // NPC) * NL + (r % NPC), :d] = arr
    return out


def kernel(**inputs):
    from concourse.bass_utils import run_bass_kernel_spmd

    src = np.asarray(inputs["src"]).astype(np.int64)
    dst = np.asarray(inputs["dst"]).astype(np.int64)
    sched, per_core = _build_schedule(src, dst)
    T = sum(nt for _, _, nt in sched)
    TA = sum(nt for _, h, nt in sched if h == 0)
    TB = T - TA

    key = ("v1", T, TA, tuple((w, h, nt) for w, h, nt in sched))
    if key not in _cache:
        _cache.clear()
        _cache[key] = _build_program(sched, TA, TB)
    nc = _cache[key]

    x = np.asarray(inputs["x"], np.float32)
    ea = np.asarray(inputs["edge_attr"], np.float32)
    x_pos = _to_pos_layout(x.astype(bf16))
    iota = np.tile(np.arange(128, dtype=np.float32)[None, :], (128, 1)).astype(bf16)
    ident = np.eye(128, dtype=np.float32).astype(bf16)

    shared = {"x_pos": x_pos, "iota": iota, "ident": ident}
    for l in range(5):
        Wm = np.asarray(inputs[f"Wm{l}"], np.float32)
        shared[f"Wmh{l}"] = Wm[:DIN_S[l]].astype(bf16)
        shared[f"Wme{l}"] = Wm[DIN_S[l]:].astype(bf16)
        shared[f"Wsb{l}"] = np.asarray(inputs[f"Ws{l}"], np.float32).astype(bf16)
        shared[f"bb{l}"] = np.asarray(inputs[f"b{l}"], np.float32).reshape(-1, 1)
    shared["Wcb"] = np.asarray(inputs["Wc"], np.float32).astype(bf16)
    shared["bcb"] = np.full((128, 1), np.asarray(inputs["bc"], np.float32).reshape(-1)[0], np.float32)

    in_maps = []
    for c in range(N_CORES):
        pc = per_core[c]
        dstw_img = pc["dstw"].reshape(T, 128).T.astype(np.float32).astype(bf16)
        ei = pc["eidx"]
        eav = np.zeros((T * 128, 8), np.float32)
        m = ei >= 0
        eav[m] = ea[ei[m]]
        ea_img = np.ascontiguousarray(
            eav.reshape(T, 128, 8).transpose(1, 0, 2).reshape(128, T * 8)).astype(bf16)
        xT_img = np.ascontiguousarray(x_pos[c * NL:(c + 1) * NL, :32].T)
        in_maps.append({
            **shared,
            "idxA": _wrap_idx(pc["posA"]),
            "idxB": _wrap_idx(pc["posB"]),
            "dstw": dstw_img,
            "eaimg": ea_img,
            "xT": xT_img,
        })

    res = run_bass_kernel_spmd(nc, in_maps, core_ids=list(range(N_CORES)))
    out = np.concatenate([res.results[c]["out"][:NPC] for c in range(N_CORES)], axis=0)
    return out.astype(np.float32)


# revision 6
# speedup vs baseline: 1.7335x; 1.1009x over previous
"""GCN message-passing (nn_Discriminator) on 8 Trainium2 NeuronCores.

Algorithm: per layer, h_new = (A@h)@Wm_h + h@Ws + segsum(edge_attr)@Wm_e + b
(the per-edge matmul commutes with segment_sum). A@h is computed per core
(nodes sharded by dst) as one-hot scatter matmuls on TensorE over edge tiles
sorted by dst window, with h rows fetched by dma_gather (4 SWDGE queues in
parallel). One-hot R matrices are host-built and streamed from HBM. Between
layers the node features are AllGathered (bf16) so every core can gather any
row.

Sharding: nodes (and edges by dst) across 8 cores; weights replicated.
"""
import numpy as np
import ml_dtypes

bf16 = ml_dtypes.bfloat16

N_NODES = 50000
N_EDGES = 800000
N_CORES = 8
NPC = N_NODES // N_CORES          # 6250 real nodes per core
P = 128
NWIN = (NPC + P - 1) // P         # 49 windows
NL = NWIN * P                     # 6272 padded nodes per core
NPOS = NL * N_CORES               # 50176 padded gather-source rows
HALF = 32768                      # int16 gather index split
CH = 64                           # gather chunk, tiles per dma_gather call
RCH = 32                          # R-stream chunk, tiles per DMA
DIMS = [32, 64, 128, 128, 128, 128]
DIN_S = DIMS[:5]
DOUT = DIMS[1:]

_cache = {}


def _build_schedule(src, dst):
    src = np.asarray(src).astype(np.int64)
    dst = np.asarray(dst).astype(np.int64)
    rank = dst // NPC
    dloc = dst % NPC
    w = dloc // P
    spos = (src // NPC) * NL + (src % NPC)
    half = (spos >= HALF).astype(np.int64)

    order = np.lexsort((spos, half, w, rank))
    rs, ws, hs = rank[order], w[order], half[order]
    counts = np.zeros((N_CORES, NWIN, 2), np.int64)
    np.add.at(counts, (rs, ws, hs), 1)
    ntiles = (-(-counts // P)).max(axis=0)       # [NWIN, 2]

    sched = []
    for wi in range(NWIN):
        for h in (0, 1):
            if ntiles[wi, h] > 0:
                sched.append((wi, h, int(ntiles[wi, h])))

    key = (rs * NWIN + ws) * 2 + hs
    boundaries = np.searchsorted(key, np.arange(N_CORES * NWIN * 2 + 1))
    per_core = []
    for c in range(N_CORES):
        posA, posB, dstw_all, eidx_all = [], [], [], []
        for (wi, h, nt) in sched:
            k = (c * NWIN + wi) * 2 + h
            lo, hi = boundaries[k], boundaries[k + 1]
            e = order[lo:hi]
            npad = nt * P - (hi - lo)
            ps = np.concatenate([spos[e] - (HALF if h else 0),
                                 np.zeros(npad, np.int64)])
            dl = np.concatenate([dloc[e] - wi * P, -np.ones(npad, np.int64)])
            ei = np.concatenate([e, -np.ones(npad, np.int64)])
            (posA if h == 0 else posB).append(ps)
            dstw_all.append(dl)
            eidx_all.append(ei)
        per_core.append(dict(
            posA=np.concatenate(posA).astype(np.int32),
            posB=np.concatenate(posB).astype(np.int32),
            dstw=np.concatenate(dstw_all).astype(np.int32),
            eidx=np.concatenate(eidx_all).astype(np.int64),
        ))
    return sched, per_core


def _wrap_idx(pos):
    n = len(pos)
    img = np.zeros((128, max(n // 16, 1)), np.int16)
    if n:
        img[np.arange(n) % 16, np.arange(n) // 16] = pos.astype(np.int16)
        for g in range(1, 8):
            img[g * 16:(g + 1) * 16] = img[:16]
    return img


def _build_program(sched, TA, TB):
    from concourse import mybir, bacc
    import concourse.tile as tile

    T = sum(nt for _, _, nt in sched)
    nc = bacc.Bacc("TRN2", target_bir_lowering=False, debug=False,
                   num_devices=N_CORES, num_swdge_queues=4)
    dt = mybir.dt

    x_pos = nc.dram_tensor("x_pos", [NPOS, 128], dt.bfloat16, kind="ExternalInput")
    xT_in = nc.dram_tensor("xT", [32, NL], dt.bfloat16, kind="ExternalInput")
    idxA_d = nc.dram_tensor("idxA", [128, TA * 8], dt.int16, kind="ExternalInput")
    idxB_d = nc.dram_tensor("idxB", [128, TB * 8], dt.int16, kind="ExternalInput")
    rimg_d = nc.dram_tensor("rimg", [128, T * 128], dt.bfloat16, kind="ExternalInput")
    ea_d = nc.dram_tensor("eaimg", [128, T * 8], dt.bfloat16, kind="ExternalInput")
    ident_d = nc.dram_tensor("ident", [128, 128], dt.bfloat16, kind="ExternalInput")
    wmh_d, ws_d, wme_d, b_d = [], [], [], []
    for l in range(5):
        wmh_d.append(nc.dram_tensor(f"Wmh{l}", [DIN_S[l], DOUT[l]], dt.bfloat16, kind="ExternalInput"))
        ws_d.append(nc.dram_tensor(f"Wsb{l}", [DIN_S[l], DOUT[l]], dt.bfloat16, kind="ExternalInput"))
        wme_d.append(nc.dram_tensor(f"Wme{l}", [8, DOUT[l]], dt.bfloat16, kind="ExternalInput"))
        b_d.append(nc.dram_tensor(f"bb{l}", [DOUT[l], 1], dt.float32, kind="ExternalInput"))
    wc_d = nc.dram_tensor("Wcb", [128, 1], dt.bfloat16, kind="ExternalInput")
    bc_d = nc.dram_tensor("bcb", [128, 1], dt.float32, kind="ExternalInput")
    out_d = nc.dram_tensor("out", [NL, 1], dt.float32, kind="ExternalOutput")

    with tile.TileContext(nc) as tc:
        with tc.tile_pool(name="consts", bufs=1) as consts, \
             tc.tile_pool(name="gxa", bufs=3) as gxa_pool, \
             tc.tile_pool(name="gxb", bufs=3) as gxb_pool, \
             tc.tile_pool(name="rst", bufs=3) as rst_pool, \
             tc.tile_pool(name="wk", bufs=3) as wk, \
             tc.tile_pool(name="hT", bufs=2) as hT_pool, \
             tc.tile_pool(name="ahp", bufs=2, space="PSUM") as ah_ps, \
             tc.tile_pool(name="aep", bufs=2, space="PSUM") as ae_ps, \
             tc.tile_pool(name="hnp", bufs=2, space="PSUM") as hn_ps, \
             tc.tile_pool(name="trp", bufs=2, space="PSUM") as tr_ps, \
             tc.tile_pool(name="dram", bufs=1, space="DRAM") as dram:

            # ---- resident constants ----
            idxA = consts.tile([128, TA * 8], dt.int16)
            idxB = consts.tile([128, TB * 8], dt.int16)
            eaim = consts.tile([128, T * 8], dt.bfloat16)
            ident = consts.tile([128, 128], dt.bfloat16)
            nc.sync.dma_start(out=idxA[:], in_=idxA_d[:])
            nc.sync.dma_start(out=idxB[:], in_=idxB_d[:])
            nc.sync.dma_start(out=eaim[:], in_=ea_d[:])
            nc.sync.dma_start(out=ident[:], in_=ident_d[:])
            wmh, wsb, wme, bb = [], [], [], []
            for l in range(5):
                t1 = consts.tile([DIN_S[l], DOUT[l]], dt.bfloat16, tag=f"wmh{l}")
                t2 = consts.tile([DIN_S[l], DOUT[l]], dt.bfloat16, tag=f"wsb{l}")
                t3 = consts.tile([8, DOUT[l]], dt.bfloat16, tag=f"wme{l}")
                t4 = consts.tile([DOUT[l], 1], dt.float32, tag=f"bb{l}")
                nc.sync.dma_start(out=t1[:], in_=wmh_d[l][:])
                nc.sync.dma_start(out=t2[:], in_=ws_d[l][:])
                nc.sync.dma_start(out=t3[:], in_=wme_d[l][:])
                nc.sync.dma_start(out=t4[:], in_=b_d[l][:])
                wmh.append(t1); wsb.append(t2); wme.append(t3); bb.append(t4)
            wc = consts.tile([128, 1], dt.bfloat16)
            bcb = consts.tile([128, 1], dt.float32)
            nc.sync.dma_start(out=wc[:], in_=wc_d[:])
            nc.sync.dma_start(out=bcb[:], in_=bc_d[:])
            xT = consts.tile([32, NL], dt.bfloat16)
            nc.sync.dma_start(out=xT[:], in_=xT_in[:])
            aeT = consts.tile([8, NL], dt.bfloat16)

            wtiles = [0] * NWIN
            for (wi, h, nt) in sched:
                wtiles[wi] += nt

            cc_in = [dram.tile([NL, 128], dt.bfloat16, tag=f"ccin{l}", name=f"ccin{l}")
                     for l in range(4)]
            cc_out = [dram.tile([NPOS, 128], dt.bfloat16, tag=f"ccout{l}", name=f"ccout{l}")
                      for l in range(4)]

            hT_cur = xT
            for l in range(5):
                din, dout = DIN_S[l], DOUT[l]
                src_dram = x_pos if l == 0 else cc_out[l - 1]
                stream_src = {"A": src_dram[:, :], "B": src_dram[HALF:, :]}
                stream_idx = {"A": idxA, "B": idxB}
                stream_T = {"A": TA, "B": TB}
                gx_pool = {"A": gxa_pool, "B": gxb_pool}
                gx_tiles = {"A": {}, "B": {}}
                r_tiles = {}
                qrr = [0]

                def slot(S, j):
                    ch = j // CH
                    tl = gx_tiles[S]
                    if ch not in tl:
                        nt = min(CH, stream_T[S] - ch * CH)
                        g = gx_pool[S].tile([128, nt, 128], dt.bfloat16, tag="gx" + S)
                        nc.gpsimd.dma_gather(
                            out_ap=g[:],
                            in_ap=stream_src[S],
                            idxs_ap=stream_idx[S][:, ch * CH * 8:(ch * CH + nt) * 8],
                            num_idxs=nt * 128,
                            num_idxs_reg=nt * 128,
                            elem_size=128,
                            single_packet=False,
                            queue_num=qrr[0],
                        )
                        qrr[0] = (qrr[0] + 1) % 4
                        tl[ch] = g
                    return tl[ch][:, j % CH, :]

                def rslot(t):
                    ch = t // RCH
                    if ch not in r_tiles:
                        nt = min(RCH, T - ch * RCH)
                        r = rst_pool.tile([128, nt * 128], dt.bfloat16, tag="rch")
                        nc.sync.dma_start(
                            out=r[:],
                            in_=rimg_d[:, ch * RCH * 128:(ch * RCH + nt) * 128])
                        r_tiles[ch] = r
                    k = t % RCH
                    return r_tiles[ch][:, k * 128:(k + 1) * 128]

                hT_next = hT_pool.tile([128, NL], dt.bfloat16, tag="hT")
                if l == 0:
                    nc.vector.memset(hT_next[64:128, :], 0)

                a_ctr = b_ctr = 0
                t_global = 0
                cur_w = -1
                done_in_w = 0
                for (wi, h, nt) in sched:
                    if wi != cur_w:
                        cur_w = wi
                        done_in_w = 0
                        ah = ah_ps.tile([din, 128], dt.float32, tag="ah", space="PSUM")
                        if l == 0:
                            ae = ae_ps.tile([8, 128], dt.float32, tag="ae", space="PSUM")
                    for j in range(nt):
                        if h == 0:
                            xs = slot("A", a_ctr); a_ctr += 1
                        else:
                            xs = slot("B", b_ctr); b_ctr += 1
                        R = rslot(t_global)
                        first = done_in_w == 0
                        last = done_in_w == wtiles[wi] - 1
                        nc.tensor.matmul(out=ah[:], lhsT=xs[:, :din], rhs=R,
                                         start=first, stop=last)
                        if l == 0:
                            nc.tensor.matmul(
                                out=ae[:], lhsT=eaim[:, t_global * 8:(t_global + 1) * 8],
                                rhs=R, start=first, stop=last)
                        done_in_w += 1
                        t_global += 1

                    if done_in_w == wtiles[wi]:
                        # ---- window epilogue ----
                        wsl = slice(wi * P, (wi + 1) * P)
                        ah_sb = wk.tile([din, 128], dt.bfloat16, tag="ah_sb")
                        nc.vector.tensor_copy(out=ah_sb[:], in_=ah[:])
                        if l == 0:
                            nc.vector.tensor_copy(out=aeT[:, wsl], in_=ae[:])
                        hn = hn_ps.tile([dout, 128], dt.float32, tag="hn", space="PSUM")
                        nc.tensor.matmul(out=hn[:], lhsT=wmh[l][:], rhs=ah_sb[:],
                                         start=True, stop=False)
                        nc.tensor.matmul(out=hn[:], lhsT=wsb[l][:],
                                         rhs=hT_cur[:din, wsl], start=False, stop=False)
                        nc.tensor.matmul(out=hn[:], lhsT=wme[l][:], rhs=aeT[:, wsl],
                                         start=False, stop=True)
                        nc.vector.tensor_scalar(
                            out=hT_next[:dout, wsl], in0=hn[:],
                            scalar1=bb[l][:], scalar2=0.0,
                            op0=mybir.AluOpType.add, op1=mybir.AluOpType.max)
                        if l < 4:
                            tr = tr_ps.tile([128, 128], dt.bfloat16, tag="tr", space="PSUM")
                            nc.tensor.transpose(out=tr[:], in_=hT_next[:, wsl], identity=ident[:])
                            rows = wk.tile([128, 128], dt.bfloat16, tag="rows")
                            nc.vector.tensor_copy(out=rows[:], in_=tr[:])
                            nc.sync.dma_start(out=cc_in[l][wsl, :], in_=rows[:])
                        else:
                            hd = tr_ps.tile([128, 1], dt.float32, tag="tr", space="PSUM")
                            nc.tensor.matmul(out=hd[:], lhsT=hT_next[:, wsl], rhs=wc[:],
                                             start=True, stop=True)
                            hd_sb = wk.tile([128, 1], dt.float32, tag="hd_sb")
                            nc.vector.tensor_scalar(
                                out=hd_sb[:], in0=hd[:], scalar1=bcb[:], scalar2=None,
                                op0=mybir.AluOpType.add)
                            nc.sync.dma_start(out=out_d[wsl, :], in_=hd_sb[:])

                if l < 4:
                    nc.gpsimd.collective_compute(
                        "AllGather",
                        mybir.AluOpType.bypass,
                        replica_groups=[list(range(N_CORES))],
                        ins=[cc_in[l].opt()],
                        outs=[cc_out[l].opt()],
                    )
                hT_cur = hT_next

    nc.finalize()
    return nc


def _to_pos_layout(arr, width=128):
    d = arr.shape[1]
    out = np.zeros((NPOS, width), arr.dtype)
    r = np.arange(N_NODES)
    out[(r // NPC) * NL + (r % NPC), :d] = arr
    return out


def kernel(**inputs):
    from concourse.bass_utils import run_bass_kernel_spmd

    src = np.asarray(inputs["src"]).astype(np.int64)
    dst = np.asarray(inputs["dst"]).astype(np.int64)
    sched, per_core = _build_schedule(src, dst)
    T = sum(nt for _, _, nt in sched)
    TA = sum(nt for _, h, nt in sched if h == 0)
    TB = T - TA

    key = ("v2", T, TA, tuple((w, h, nt) for w, h, nt in sched))
    if key not in _cache:
        _cache.clear()
        _cache[key] = _build_program(sched, TA, TB)
    nc = _cache[key]

    x = np.asarray(inputs["x"], np.float32)
    ea = np.asarray(inputs["edge_attr"], np.float32)
    x_pos = _to_pos_layout(x.astype(bf16))
    ident = np.eye(128, dtype=np.float32).astype(bf16)

    shared = {"x_pos": x_pos, "ident": ident}
    for l in range(5):
        Wm = np.asarray(inputs[f"Wm{l}"], np.float32)
        shared[f"Wmh{l}"] = Wm[:DIN_S[l]].astype(bf16)
        shared[f"Wme{l}"] = Wm[DIN_S[l]:].astype(bf16)
        shared[f"Wsb{l}"] = np.asarray(inputs[f"Ws{l}"], np.float32).astype(bf16)
        shared[f"bb{l}"] = np.asarray(inputs[f"b{l}"], np.float32).reshape(-1, 1)
    shared["Wcb"] = np.asarray(inputs["Wc"], np.float32).astype(bf16)
    shared["bcb"] = np.full((128, 1), np.asarray(inputs["bc"], np.float32).reshape(-1)[0], np.float32)

    in_maps = []
    for c in range(N_CORES):
        pc = per_core[c]
        dstw = pc["dstw"]
        # one-hot R image [128, T*128]: R[p, t*128+n] = (dstw[t*128+p] == n)
        rimg = np.zeros((128, T * 128), bf16)
        sl = np.arange(T * 128)
        m = dstw >= 0
        rimg[sl[m] % 128, (sl[m] // 128) * 128 + dstw[m]] = 1
        ei = pc["eidx"]
        eav = np.zeros((T * 128, 8), np.float32)
        me = ei >= 0
        eav[me] = ea[ei[me]]
        ea_img = np.ascontiguousarray(
            eav.reshape(T, 128, 8).transpose(1, 0, 2).reshape(128, T * 8)).astype(bf16)
        xT_img = np.ascontiguousarray(x_pos[c * NL:(c + 1) * NL, :32].T)
        in_maps.append({
            **shared,
            "idxA": _wrap_idx(pc["posA"]),
            "idxB": _wrap_idx(pc["posB"]),
            "rimg": rimg,
            "eaimg": ea_img,
            "xT": xT_img,
        })

    res = run_bass_kernel_spmd(nc, in_maps, core_ids=list(range(N_CORES)))
    out = np.concatenate([res.results[c]["out"][:NPC] for c in range(N_CORES)], axis=0)
    return out.astype(np.float32)


# revision 7
# speedup vs baseline: 1.9702x; 1.1366x over previous
"""GCN message-passing (nn_Discriminator) on 8 Trainium2 NeuronCores.

Algorithm: per layer, h_new = (A@h)@Wm_h + h@Ws + segsum(edge_attr)@Wm_e + b
(the per-edge matmul commutes with segment_sum). A@h is computed per core
(nodes sharded by dst) as one-hot scatter matmuls on TensorE over edge tiles
sorted by dst window, with h rows fetched by dma_gather (4 SWDGE queues in
parallel). One-hot R matrices are host-built and streamed from HBM. Between
layers the node features are AllGathered (bf16) so every core can gather any
row.

Sharding: nodes (and edges by dst) across 8 cores; weights replicated.
"""
import numpy as np
import ml_dtypes

bf16 = ml_dtypes.bfloat16

N_NODES = 50000
N_EDGES = 800000
N_CORES = 8
NPC = N_NODES // N_CORES          # 6250 real nodes per core
P = 128
NWIN = (NPC + P - 1) // P         # 49 windows
NL = NWIN * P                     # 6272 padded nodes per core
NPOS = NL * N_CORES               # 50176 padded gather-source rows
HALF = 32768                      # int16 gather index split
CH = 32                           # gather chunk, tiles per dma_gather call
RCH = 32                          # R-stream chunk, tiles per DMA
DIMS = [32, 64, 128, 128, 128, 128]
DIN_S = DIMS[:5]
DOUT = DIMS[1:]

_cache = {}


def _build_schedule(src, dst):
    src = np.asarray(src).astype(np.int64)
    dst = np.asarray(dst).astype(np.int64)
    rank = dst // NPC
    dloc = dst % NPC
    w = dloc // P
    spos = (src // NPC) * NL + (src % NPC)
    half = (spos >= HALF).astype(np.int64)

    order = np.lexsort((spos, half, w, rank))
    rs, ws, hs = rank[order], w[order], half[order]
    counts = np.zeros((N_CORES, NWIN, 2), np.int64)
    np.add.at(counts, (rs, ws, hs), 1)
    ntiles = (-(-counts // P)).max(axis=0)       # [NWIN, 2]

    sched = []
    for wi in range(NWIN):
        for h in (0, 1):
            if ntiles[wi, h] > 0:
                sched.append((wi, h, int(ntiles[wi, h])))

    key = (rs * NWIN + ws) * 2 + hs
    boundaries = np.searchsorted(key, np.arange(N_CORES * NWIN * 2 + 1))
    per_core = []
    for c in range(N_CORES):
        posA, posB, dstw_all, eidx_all = [], [], [], []
        for (wi, h, nt) in sched:
            k = (c * NWIN + wi) * 2 + h
            lo, hi = boundaries[k], boundaries[k + 1]
            e = order[lo:hi]
            npad = nt * P - (hi - lo)
            ps = np.concatenate([spos[e] - (HALF if h else 0),
                                 np.zeros(npad, np.int64)])
            dl = np.concatenate([dloc[e] - wi * P, -np.ones(npad, np.int64)])
            ei = np.concatenate([e, -np.ones(npad, np.int64)])
            (posA if h == 0 else posB).append(ps)
            dstw_all.append(dl)
            eidx_all.append(ei)
        per_core.append(dict(
            posA=np.concatenate(posA).astype(np.int32),
            posB=np.concatenate(posB).astype(np.int32),
            dstw=np.concatenate(dstw_all).astype(np.int32),
            eidx=np.concatenate(eidx_all).astype(np.int64),
        ))
    return sched, per_core


def _wrap_idx(pos):
    n = len(pos)
    img = np.zeros((128, max(n // 16, 1)), np.int16)
    if n:
        img[np.arange(n) % 16, np.arange(n) // 16] = pos.astype(np.int16)
        for g in range(1, 8):
            img[g * 16:(g + 1) * 16] = img[:16]
    return img


def _build_program(sched, TA, TB):
    from concourse import mybir, bacc
    import concourse.tile as tile

    T = sum(nt for _, _, nt in sched)
    nc = bacc.Bacc("TRN2", target_bir_lowering=False, debug=False,
                   num_devices=N_CORES, num_swdge_queues=4)
    dt = mybir.dt

    x_pos = nc.dram_tensor("x_pos", [NPOS, 128], dt.bfloat16, kind="ExternalInput")
    xT_in = nc.dram_tensor("xT", [32, NL], dt.bfloat16, kind="ExternalInput")
    idxA_d = nc.dram_tensor("idxA", [128, TA * 8], dt.int16, kind="ExternalInput")
    idxB_d = nc.dram_tensor("idxB", [128, TB * 8], dt.int16, kind="ExternalInput")
    rimg_d = nc.dram_tensor("rimg", [128, T * 128], dt.bfloat16, kind="ExternalInput")
    ea_d = nc.dram_tensor("eaimg", [128, T * 8], dt.bfloat16, kind="ExternalInput")
    ident_d = nc.dram_tensor("ident", [128, 128], dt.bfloat16, kind="ExternalInput")
    wmh_d, ws_d, wme_d, b_d = [], [], [], []
    for l in range(5):
        wmh_d.append(nc.dram_tensor(f"Wmh{l}", [DIN_S[l], DOUT[l]], dt.bfloat16, kind="ExternalInput"))
        ws_d.append(nc.dram_tensor(f"Wsb{l}", [DIN_S[l], DOUT[l]], dt.bfloat16, kind="ExternalInput"))
        wme_d.append(nc.dram_tensor(f"Wme{l}", [8, DOUT[l]], dt.bfloat16, kind="ExternalInput"))
        b_d.append(nc.dram_tensor(f"bb{l}", [DOUT[l], 1], dt.float32, kind="ExternalInput"))
    wc_d = nc.dram_tensor("Wcb", [128, 1], dt.bfloat16, kind="ExternalInput")
    bc_d = nc.dram_tensor("bcb", [128, 1], dt.float32, kind="ExternalInput")
    out_d = nc.dram_tensor("out", [NL, 1], dt.float32, kind="ExternalOutput")

    with tile.TileContext(nc) as tc:
        with tc.tile_pool(name="consts", bufs=1) as consts, \
             tc.tile_pool(name="gxa", bufs=5) as gxa_pool, \
             tc.tile_pool(name="gxb", bufs=5) as gxb_pool, \
             tc.tile_pool(name="rst", bufs=3) as rst_pool, \
             tc.tile_pool(name="wk", bufs=3) as wk, \
             tc.tile_pool(name="hT", bufs=2) as hT_pool, \
             tc.tile_pool(name="ahp", bufs=2, space="PSUM") as ah_ps, \
             tc.tile_pool(name="aep", bufs=2, space="PSUM") as ae_ps, \
             tc.tile_pool(name="hnp", bufs=2, space="PSUM") as hn_ps, \
             tc.tile_pool(name="trp", bufs=2, space="PSUM") as tr_ps, \
             tc.tile_pool(name="dram", bufs=1, space="DRAM") as dram:

            # ---- resident constants ----
            idxA = consts.tile([128, TA * 8], dt.int16)
            idxB = consts.tile([128, TB * 8], dt.int16)
            eaim = consts.tile([128, T * 8], dt.bfloat16)
            ident = consts.tile([128, 128], dt.bfloat16)
            nc.sync.dma_start(out=idxA[:], in_=idxA_d[:])
            nc.sync.dma_start(out=idxB[:], in_=idxB_d[:])
            nc.sync.dma_start(out=eaim[:], in_=ea_d[:])
            nc.sync.dma_start(out=ident[:], in_=ident_d[:])
            wmh, wsb, wme, bb = [], [], [], []
            for l in range(5):
                t1 = consts.tile([DIN_S[l], DOUT[l]], dt.bfloat16, tag=f"wmh{l}")
                t2 = consts.tile([DIN_S[l], DOUT[l]], dt.bfloat16, tag=f"wsb{l}")
                t3 = consts.tile([8, DOUT[l]], dt.bfloat16, tag=f"wme{l}")
                t4 = consts.tile([DOUT[l], 1], dt.float32, tag=f"bb{l}")
                nc.sync.dma_start(out=t1[:], in_=wmh_d[l][:])
                nc.sync.dma_start(out=t2[:], in_=ws_d[l][:])
                nc.sync.dma_start(out=t3[:], in_=wme_d[l][:])
                nc.sync.dma_start(out=t4[:], in_=b_d[l][:])
                wmh.append(t1); wsb.append(t2); wme.append(t3); bb.append(t4)
            wc = consts.tile([128, 1], dt.bfloat16)
            bcb = consts.tile([128, 1], dt.float32)
            nc.sync.dma_start(out=wc[:], in_=wc_d[:])
            nc.sync.dma_start(out=bcb[:], in_=bc_d[:])
            xT = consts.tile([32, NL], dt.bfloat16)
            nc.sync.dma_start(out=xT[:], in_=xT_in[:])
            aeT = consts.tile([8, NL], dt.bfloat16)

            wtiles = [0] * NWIN
            for (wi, h, nt) in sched:
                wtiles[wi] += nt

            cc_in = [dram.tile([NL, 128], dt.bfloat16, tag=f"ccin{l}", name=f"ccin{l}")
                     for l in range(4)]
            cc_out = [dram.tile([NPOS, 128], dt.bfloat16, tag=f"ccout{l}", name=f"ccout{l}")
                      for l in range(4)]

            hT_cur = xT
            for l in range(5):
                din, dout = DIN_S[l], DOUT[l]
                src_dram = x_pos if l == 0 else cc_out[l - 1]
                stream_src = {"A": src_dram[:, :], "B": src_dram[HALF:, :]}
                stream_idx = {"A": idxA, "B": idxB}
                stream_T = {"A": TA, "B": TB}
                gx_pool = {"A": gxa_pool, "B": gxb_pool}
                gx_tiles = {"A": {}, "B": {}}
                r_tiles = {}
                qrr = [0]

                def slot(S, j):
                    ch = j // CH
                    tl = gx_tiles[S]
                    if ch not in tl:
                        nt = min(CH, stream_T[S] - ch * CH)
                        g = gx_pool[S].tile([128, nt, 128], dt.bfloat16, tag="gx" + S)
                        nc.gpsimd.dma_gather(
                            out_ap=g[:],
                            in_ap=stream_src[S],
                            idxs_ap=stream_idx[S][:, ch * CH * 8:(ch * CH + nt) * 8],
                            num_idxs=nt * 128,
                            num_idxs_reg=nt * 128,
                            elem_size=128,
                            single_packet=False,
                            queue_num=qrr[0],
                        )
                        qrr[0] = (qrr[0] + 1) % 4
                        tl[ch] = g
                    return tl[ch][:, j % CH, :]

                def rslot(t):
                    ch = t // RCH
                    if ch not in r_tiles:
                        nt = min(RCH, T - ch * RCH)
                        r = rst_pool.tile([128, nt * 128], dt.bfloat16, tag="rch")
                        nc.sync.dma_start(
                            out=r[:],
                            in_=rimg_d[:, ch * RCH * 128:(ch * RCH + nt) * 128])
                        r_tiles[ch] = r
                    k = t % RCH
                    return r_tiles[ch][:, k * 128:(k + 1) * 128]

                hT_next = hT_pool.tile([128, NL], dt.bfloat16, tag="hT")
                if l == 0:
                    nc.vector.memset(hT_next[64:128, :], 0)

                a_ctr = b_ctr = 0
                t_global = 0
                cur_w = -1
                done_in_w = 0
                for (wi, h, nt) in sched:
                    if wi != cur_w:
                        cur_w = wi
                        done_in_w = 0
                        ah = ah_ps.tile([din, 128], dt.float32, tag="ah", space="PSUM")
                        if l == 0:
                            ae = ae_ps.tile([8, 128], dt.float32, tag="ae", space="PSUM")
                    for j in range(nt):
                        if h == 0:
                            xs = slot("A", a_ctr); a_ctr += 1
                        else:
                            xs = slot("B", b_ctr); b_ctr += 1
                        R = rslot(t_global)
                        first = done_in_w == 0
                        last = done_in_w == wtiles[wi] - 1
                        nc.tensor.matmul(out=ah[:], lhsT=xs[:, :din], rhs=R,
                                         start=first, stop=last)
                        if l == 0:
                            nc.tensor.matmul(
                                out=ae[:], lhsT=eaim[:, t_global * 8:(t_global + 1) * 8],
                                rhs=R, start=first, stop=last)
                        done_in_w += 1
                        t_global += 1

                    if done_in_w == wtiles[wi]:
                        # ---- window epilogue ----
                        wsl = slice(wi * P, (wi + 1) * P)
                        ah_sb = wk.tile([din, 128], dt.bfloat16, tag="ah_sb")
                        nc.vector.tensor_copy(out=ah_sb[:], in_=ah[:])
                        if l == 0:
                            nc.vector.tensor_copy(out=aeT[:, wsl], in_=ae[:])
                        hn = hn_ps.tile([dout, 128], dt.float32, tag="hn", space="PSUM")
                        nc.tensor.matmul(out=hn[:], lhsT=wmh[l][:], rhs=ah_sb[:],
                                         start=True, stop=False)
                        nc.tensor.matmul(out=hn[:], lhsT=wsb[l][:],
                                         rhs=hT_cur[:din, wsl], start=False, stop=False)
                        nc.tensor.matmul(out=hn[:], lhsT=wme[l][:], rhs=aeT[:, wsl],
                                         start=False, stop=True)
                        nc.vector.tensor_scalar(
                            out=hT_next[:dout, wsl], in0=hn[:],
                            scalar1=bb[l][:], scalar2=0.0,
                            op0=mybir.AluOpType.add, op1=mybir.AluOpType.max)
                        if l < 4:
                            tr = tr_ps.tile([128, 128], dt.bfloat16, tag="tr", space="PSUM")
                            nc.tensor.transpose(out=tr[:], in_=hT_next[:, wsl], identity=ident[:])
                            rows = wk.tile([128, 128], dt.bfloat16, tag="rows")
                            nc.vector.tensor_copy(out=rows[:], in_=tr[:])
                            nc.sync.dma_start(out=cc_in[l][wsl, :], in_=rows[:])
                        else:
                            hd = tr_ps.tile([128, 1], dt.float32, tag="tr", space="PSUM")
                            nc.tensor.matmul(out=hd[:], lhsT=hT_next[:, wsl], rhs=wc[:],
                                             start=True, stop=True)
                            hd_sb = wk.tile([128, 1], dt.float32, tag="hd_sb")
                            nc.vector.tensor_scalar(
                                out=hd_sb[:], in0=hd[:], scalar1=bcb[:], scalar2=None,
                                op0=mybir.AluOpType.add)
                            nc.sync.dma_start(out=out_d[wsl, :], in_=hd_sb[:])

                if l < 4:
                    nc.gpsimd.collective_compute(
                        "AllGather",
                        mybir.AluOpType.bypass,
                        replica_groups=[list(range(N_CORES))],
                        ins=[cc_in[l].opt()],
                        outs=[cc_out[l].opt()],
                    )
                hT_cur = hT_next

    nc.finalize()
    return nc


def _to_pos_layout(arr, width=128):
    d = arr.shape[1]
    out = np.zeros((NPOS, width), arr.dtype)
    r = np.arange(N_NODES)
    out[(r // NPC) * NL + (r % NPC), :d] = arr
    return out


def kernel(**inputs):
    from concourse.bass_utils import run_bass_kernel_spmd

    src = np.asarray(inputs["src"]).astype(np.int64)
    dst = np.asarray(inputs["dst"]).astype(np.int64)
    sched, per_core = _build_schedule(src, dst)
    T = sum(nt for _, _, nt in sched)
    TA = sum(nt for _, h, nt in sched if h == 0)
    TB = T - TA

    key = ("v2b", T, TA, tuple((w, h, nt) for w, h, nt in sched))
    if key not in _cache:
        _cache.clear()
        _cache[key] = _build_program(sched, TA, TB)
    nc = _cache[key]

    x = np.asarray(inputs["x"], np.float32)
    ea = np.asarray(inputs["edge_attr"], np.float32)
    x_pos = _to_pos_layout(x.astype(bf16))
    ident = np.eye(128, dtype=np.float32).astype(bf16)

    shared = {"x_pos": x_pos, "ident": ident}
    for l in range(5):
        Wm = np.asarray(inputs[f"Wm{l}"], np.float32)
        shared[f"Wmh{l}"] = Wm[:DIN_S[l]].astype(bf16)
        shared[f"Wme{l}"] = Wm[DIN_S[l]:].astype(bf16)
        shared[f"Wsb{l}"] = np.asarray(inputs[f"Ws{l}"], np.float32).astype(bf16)
        shared[f"bb{l}"] = np.asarray(inputs[f"b{l}"], np.float32).reshape(-1, 1)
    shared["Wcb"] = np.asarray(inputs["Wc"], np.float32).astype(bf16)
    shared["bcb"] = np.full((128, 1), np.asarray(inputs["bc"], np.float32).reshape(-1)[0], np.float32)

    in_maps = []
    for c in range(N_CORES):
        pc = per_core[c]
        dstw = pc["dstw"]
        # one-hot R image [128, T*128]: R[p, t*128+n] = (dstw[t*128+p] == n)
        rimg = np.zeros((128, T * 128), bf16)
        sl = np.arange(T * 128)
        m = dstw >= 0
        rimg[sl[m] % 128, (sl[m] // 128) * 128 + dstw[m]] = 1
        ei = pc["eidx"]
        eav = np.zeros((T * 128, 8), np.float32)
        me = ei >= 0
        eav[me] = ea[ei[me]]
        ea_img = np.ascontiguousarray(
            eav.reshape(T, 128, 8).transpose(1, 0, 2).reshape(128, T * 8)).astype(bf16)
        xT_img = np.ascontiguousarray(x_pos[c * NL:(c + 1) * NL, :32].T)
        in_maps.append({
            **shared,
            "idxA": _wrap_idx(pc["posA"]),
            "idxB": _wrap_idx(pc["posB"]),
            "rimg": rimg,
            "eaimg": ea_img,
            "xT": xT_img,
        })

    res = run_bass_kernel_spmd(nc, in_maps, core_ids=list(range(N_CORES)))
    out = np.concatenate([res.results[c]["out"][:NPC] for c in range(N_CORES)], axis=0)
    return out.astype(np.float32)
